# revision 3
# baseline (speedup 1.0000x reference)
"""GAT-style bipartite graph attention layer (nn_BiGraphContrastLayer) on 8 trn2
cores — v2.2 (549-558 us vs 694 us baseline, rel err 0.0083).

Key facts (measured on HW):
  - DRAM "Shared" scratchpad is shared between core PAIRS {2k, 2k+1} only.
  - SWDGE gathers on one queue drain ~7.7 ns/row; 4 queues give ~390 GB/s.
  - Matmul: 512-col bf16 stream ~380 ns; tiny matmuls floor ~165 ns;
    LDWEIGHTS hides under pipelined streams.
  - Broadcast DVE mult runs ~2x slower than dense; split into 2 chunk-range
    instructions so PE segment-sums can start early.  fp8 for msg/sel was
    tried and REJECTED: rel err 0.065 > 2e-2 gate and no speedup.

Design:
  - Pair-shared zel table [NPAD, 640]: rows = all 10000 src nodes + the
    pair's own 2500 dst nodes (self-loops/era only touch own dsts).  Each
    core computes 50 subtiles of 128 rows (3 uniform runs: 40 src + 9+1 own
    dst, so er of own dsts stays local) at per-core register row offsets
    (dynamic DRAM dma_start).
  - Pairwise AllGather (tiny) as the cross-core barrier.
  - Phase 2: per dst tile, 4 sub-gathers rotated across the 4 SWDGE queues;
    attention + segment-sum matmuls as before.
"""
import os

import numpy as np
import ml_dtypes

import concourse.bacc as bacc
import concourse.bass as bass
import concourse.mybir as mybir
import concourse.tile as tile
from concourse.bass import ds

BF = ml_dtypes.bfloat16
F8 = ml_dtypes.float8_e4m3fn
F32 = np.float32

NS, ND, E, DIN, H, DH = 10000, 10000, 320000, 512, 8, 64
NEG = 0.2
NCORES = 8
DPC = ND // NCORES          # 1250 dst nodes per core
N = NS + ND
NPAD = 20480
ROW = 640                   # zel row elems: z(512) | el(8) | er(8) | pad(112)
NTILES = 10                 # dst tiles of 128 per core (1280 >= 1250)
RUNS = (40, 9, 1)           # src run, dst run, dst tail (x128 rows)
MSUB = sum(RUNS)            # 50


# ----------------------------------------------------------------- host prep
def _wrap_idx(idx):
    k = len(idx)
    w = np.zeros((16, k // 16), np.int16)
    w[np.arange(k) % 16, np.arange(k) // 16] = idx
    return np.tile(w, (8, 1))


def _host_prep(x_src, x_dst, edge_src, edge_dst, W, attn_l, attn_r, bias):
    xcat = np.zeros((NPAD, DIN), F32)
    xcat[:NS] = x_src
    xcat[NS:N] = x_dst
    Al = np.zeros((DIN, H), F32)
    Ar = np.zeros((DIN, H), F32)
    for h in range(H):
        Al[h * DH:(h + 1) * DH, h] = attn_l[h]
        Ar[h * DH:(h + 1) * DH, h] = attn_r[h]
    Wext = np.concatenate([W, W @ Al, W @ Ar], 1).astype(BF)  # [512, 528]
    bias_rep = np.tile(bias[None, :].astype(F32), (128, 1))   # [128, 512]

    edge_src = edge_src.astype(np.int64)
    edge_dst = edge_dst.astype(np.int64)
    tlists = [[None] * NTILES for _ in range(NCORES)]
    kmax = 0
    for c in range(NCORES):
        d0 = c * DPC
        m = (edge_dst >= d0) & (edge_dst < d0 + DPC)
        es = np.concatenate([edge_src[m],
                             NS + d0 + np.arange(DPC, dtype=np.int64)])
        ed = np.concatenate([edge_dst[m] - d0, np.arange(DPC, dtype=np.int64)])
        order = np.argsort(ed, kind="stable")
        es, ed = es[order], ed[order]
        for t in range(NTILES):
            sel = (ed >= t * 128) & (ed < (t + 1) * 128)
            tlists[c][t] = (es[sel], ed[sel] - t * 128)
            kmax = max(kmax, int(sel.sum()))
    k_tile = ((kmax + 127) // 128) * 128
    nch = k_tile // 128

    per_core = []
    for c in range(NCORES):
        zidx = np.zeros((128, NTILES * k_tile // 16), np.int16)
        selT = np.zeros((128, NTILES * nch * 128), BF)
        selD = np.zeros((128, NTILES * nch * 128), BF)
        for t in range(NTILES):
            es, edl = tlists[c][t]
            k = len(es)
            src = np.zeros(k_tile, np.int64)
            src[:k] = es
            s16 = slice(t * k_tile // 16, (t + 1) * k_tile // 16)
            zidx[:, s16] = _wrap_idx(src)
            dstl = np.full(k_tile, -1, np.int64)
            dstl[:k] = edl
            for ch in range(nch):
                dl = dstl[ch * 128:(ch + 1) * 128]
                sm = np.zeros((128, 128), F32)
                valid = dl >= 0
                sm[np.arange(128)[valid], dl[valid]] = 1.0
                j = (t * nch + ch) * 128
                selT[:, j:j + 128] = sm.astype(BF)
                selD[:, j:j + 128] = sm.T.astype(BF)

        # phase-1 shard: 3 runs of subtiles; row bases depend on core parity
        d0 = NS + c * DPC
        if c % 2 == 0:
            bases = [0, d0, d0 + RUNS[1] * 128]
        else:
            bases = [RUNS[0] * 128, d0, d0 + RUNS[1] * 128]
        rows = np.concatenate(
            [np.arange(b, b + r * 128) for b, r in zip(bases, RUNS)])
        rows = np.clip(rows, 0, NPAD - 1)
        xs = xcat[rows]                               # [6400, 512]
        xTs = np.zeros((128, 4, MSUB * 128), BF)
        for k4 in range(4):
            xTs[:, k4, :] = xs[:, k4 * 128:(k4 + 1) * 128].T.astype(BF)
        per_core.append(dict(selT=selT, selD=selD, zidx=zidx,
                             xTs=xTs.reshape(128, 4 * MSUB * 128),
                             bases=np.array([bases], np.int32)))

    shared = dict(Wext=Wext, bias_rep=bias_rep)
    return shared, per_core, k_tile, nch


# ------------------------------------------------------------- bass program
def _build_nc(k_tile, nch):
    nc = bacc.Bacc("TRN2", target_bir_lowering=False, debug=False,
                   num_devices=NCORES, num_swdge_queues=4)
    dt = mybir.dt
    groups = [[2 * p, 2 * p + 1] for p in range(NCORES // 2)]

    W_d = nc.dram_tensor("Wext", [DIN, 528], dt.bfloat16, kind="ExternalInput")
    bias_d = nc.dram_tensor("bias_rep", [128, 512], dt.float32,
                            kind="ExternalInput")
    xTs_d = nc.dram_tensor("xTs", [128, 4 * MSUB * 128], dt.bfloat16,
                           kind="ExternalInput")
    bases_d = nc.dram_tensor("bases", [1, 3], dt.int32, kind="ExternalInput")
    selT_d = nc.dram_tensor("selT", [128, NTILES * nch * 128], dt.bfloat16,
                            kind="ExternalInput")
    selD_d = nc.dram_tensor("selD", [128, NTILES * nch * 128], dt.bfloat16,
                            kind="ExternalInput")
    zidx_d = nc.dram_tensor("zidx", [128, NTILES * k_tile // 16], dt.int16,
                            kind="ExternalInput")
    out_d = nc.dram_tensor("out", [NTILES * 128, 512], dt.float32,
                           kind="ExternalOutput")
    zel_d = nc.dram_tensor("zel_tab", [NPAD, ROW], dt.bfloat16,
                           addr_space="Shared")
    bar_i = nc.dram_tensor("bar_i", [1, 64], dt.int32)
    bar_o = nc.dram_tensor("bar_o", [2, 64], dt.int32)

    with tile.TileContext(nc) as tc:
        with tc.tile_pool(name="const", bufs=1) as cpool:
            wsb = cpool.tile([128, 4 * 528], dt.bfloat16)
            for k in range(4):
                nc.sync.dma_start(wsb[:, k * 528:(k + 1) * 528],
                                  W_d[k * 128:(k + 1) * 128, :])
            bias_sb = cpool.tile([128, 512], dt.float32)
            nc.sync.dma_start(bias_sb[:], bias_d[:])
            zidx_sb = cpool.tile([128, NTILES * k_tile // 16], dt.int16)
            nc.sync.dma_start(zidx_sb[:], zidx_d[:])
            era = cpool.tile([128, NTILES, 8], dt.bfloat16)
            base_sb = cpool.tile([1, 3], dt.int32)
            nc.sync.dma_start(base_sb[:], bases_d[:])
            bar_sb = cpool.tile([1, 64], dt.int32)
            nc.vector.memset(bar_sb[:], 0)
            nc.sync.dma_start(bar_i[:], bar_sb[:])

            rbase = [nc.values_load(base_sb[0:1, i:i + 1], min_val=0,
                                    max_val=NPAD - RUNS[i] * 128,
                                    skip_runtime_bounds_check=True)
                     for i in range(3)]

            # ---- phase 1: this core's zel rows -> pair-shared table
            with (
                tc.tile_pool(name="xp", bufs=1) as xpool,
                tc.tile_pool(name="zel", bufs=3) as zpool,
                tc.tile_pool(name="p1", bufs=2, space="PSUM") as p1pool,
                tc.tile_pool(name="p1b", bufs=2, space="PSUM") as p1bpool,
            ):
                xp = xpool.tile([128, 4, MSUB * 128], dt.bfloat16)
                nc.sync.dma_start(
                    xp[:], xTs_d[:].rearrange("p (k m) -> p k m", k=4))
                m = 0
                for run, base in zip(RUNS, rbase):
                    for i in range(run):
                        zps = p1pool.tile([128, 512], dt.float32,
                                          space="PSUM", name="zps")
                        lps = p1bpool.tile([128, 16], dt.float32,
                                           space="PSUM", name="lps")
                        for k in range(4):
                            lhsT = xp[:, k, m * 128:(m + 1) * 128]
                            nc.tensor.matmul(zps[:], lhsT,
                                             wsb[:, k * 528:k * 528 + 512],
                                             start=(k == 0), stop=(k == 3))
                            nc.tensor.matmul(
                                lps[:], lhsT,
                                wsb[:, k * 528 + 512:(k + 1) * 528],
                                start=(k == 0), stop=(k == 3))
                        zel_sb = zpool.tile([128, ROW], dt.bfloat16,
                                            name="zel_sb")
                        nc.scalar.activation(
                            zel_sb[:, 0:512], zps[:],
                            mybir.ActivationFunctionType.Copy)
                        nc.vector.tensor_copy(zel_sb[:, 512:528], lps[:])
                        if m >= RUNS[0]:
                            nc.vector.tensor_copy(era[:, m - RUNS[0], :],
                                                  lps[:, 8:16])
                        if m < 3:
                            nc.gpsimd.memset(zel_sb[:, 528:ROW], 0)
                        nc.sync.dma_start(zel_d[ds(base + i * 128, 128), :],
                                          zel_sb[:])
                        m += 1

            # ---- pairwise barrier
            tc.strict_bb_all_engine_barrier()
            nc.gpsimd.collective_compute(
                "AllGather", mybir.AluOpType.bypass, groups,
                ins=[bar_i[:].rearrange("a b -> (a b)")],
                outs=[bar_o[:].rearrange("a b -> (a b)")])
            tc.strict_bb_all_engine_barrier()

            # ---- phase 2
            sub = [0]
            for i in range(4):
                sub.append(sub[-1] + (nch + 3 - i) // 4)
            with (
                tc.tile_pool(name="zg", bufs=3) as zgpool,
                tc.tile_pool(name="sel", bufs=2) as selpool,
                tc.tile_pool(name="seld", bufs=2) as seldpool,
                tc.tile_pool(name="sc", bufs=3) as scpool,
                tc.tile_pool(name="eo", bufs=2) as eopool,
                tc.tile_pool(name="p2", bufs=2, space="PSUM") as p2pool,
                tc.tile_pool(name="p2b", bufs=2, space="PSUM") as p2bpool,
                tc.tile_pool(name="p2c", bufs=2, space="PSUM") as p2cpool,
            ):
                for t in range(NTILES):
                    zg = zgpool.tile([128, nch, ROW], dt.bfloat16, name="zg")
                    for j in range(4):
                        c0, c1 = sub[j], sub[j + 1]
                        i16 = slice((t * nch + c0) * 8, (t * nch + c1) * 8)
                        nc.gpsimd.dma_gather(
                            zg[:, c0:c1, :], zel_d[:], zidx_sb[:, i16],
                            num_idxs=(c1 - c0) * 128,
                            num_idxs_reg=(c1 - c0) * 128,
                            elem_size=ROW, single_packet=False,
                            queue_num=j)
                    sel = selpool.tile([128, nch * 128], dt.bfloat16,
                                       name="sel")
                    nc.sync.dma_start(
                        sel[:], selT_d[:, t * nch * 128:(t + 1) * nch * 128])
                    seld = seldpool.tile([128, nch * 128], dt.bfloat16,
                                         name="seld")
                    nc.sync.dma_start(
                        seld[:], selD_d[:, t * nch * 128:(t + 1) * nch * 128])

                    lt = scpool.tile([128, nch, 8], dt.float32, tag="lt",
                                     name="lt")
                    pe_er = p2cpool.tile([128, nch, 8], dt.float32,
                                         space="PSUM", name="pe_er")
                    for ch in range(nch):
                        nc.tensor.matmul(pe_er[:, ch, :],
                                         seld[:, ch * 128:(ch + 1) * 128],
                                         era[:, t, :],
                                         start=True, stop=True,
                                         skip_group_check=True)
                    nc.vector.tensor_tensor(
                        lt[:], zg[:, :, 512:520], pe_er[:],
                        op=mybir.AluOpType.add)
                    nc.vector.scalar_tensor_tensor(
                        lt[:], lt[:], NEG, lt[:],
                        op0=mybir.AluOpType.mult, op1=mybir.AluOpType.max)
                    vb = scpool.tile([128, nch, 8], dt.bfloat16, tag="vb",
                                     name="vb")
                    nc.scalar.activation(vb[:], lt[:],
                                         mybir.ActivationFunctionType.Exp)

                    po = p2pool.tile([128, 512], dt.float32, space="PSUM",
                                     name="po")
                    ps = p2bpool.tile([128, 8], dt.float32, space="PSUM",
                                      name="ps")
                    # msg = v * z split into 2 chunk ranges so segment-sum
                    # matmuls start while the second range still multiplies
                    for j in range(2):
                        c0, c1 = sub[2 * j], sub[2 * j + 2]
                        z4 = zg[:, c0:c1, 0:512].rearrange(
                            "p c (h d) -> p c h d", d=DH)
                        nc.vector.tensor_tensor(
                            z4, z4,
                            vb[:, c0:c1, :].to_broadcast(
                                [128, c1 - c0, 8, DH]),
                            op=mybir.AluOpType.mult)
                        for ch in range(c0, c1):
                            sl = sel[:, ch * 128:(ch + 1) * 128]
                            nc.tensor.matmul(po[:], sl, zg[:, ch, 0:512],
                                             start=(ch == 0),
                                             stop=(ch == nch - 1))
                            nc.tensor.matmul(ps[:], sl, vb[:, ch, :],
                                             start=(ch == 0),
                                             stop=(ch == nch - 1))

                    ssb = scpool.tile([128, 8], dt.float32, tag="ssb",
                                      name="ssb")
                    nc.vector.tensor_scalar_add(ssb[:], ps[:], 1e-30)
                    nc.vector.reciprocal(ssb[:], ssb[:])
                    osb = eopool.tile([128, 512], dt.float32, name="osb")
                    o4 = osb[:].rearrange("p (h d) -> p h d", d=DH)
                    nc.vector.tensor_tensor(
                        o4, po[:].rearrange("p (h d) -> p h d", d=DH),
                        ssb[:].to_broadcast([128, 8, DH]),
                        op=mybir.AluOpType.mult)
                    nc.vector.tensor_tensor(osb[:], osb[:], bias_sb[:],
                                            op=mybir.AluOpType.add)
                    nc.sync.dma_start(out_d[t * 128:(t + 1) * 128, :], osb[:])
    nc.compile()
    return nc


# ------------------------------------------------------------------- driver
def kernel(x_src, x_dst, edge_src, edge_dst, W, attn_l, attn_r, bias):
    shared, per_core, k_tile, nch = _host_prep(
        np.asarray(x_src), np.asarray(x_dst), np.asarray(edge_src),
        np.asarray(edge_dst), np.asarray(W), np.asarray(attn_l),
        np.asarray(attn_r), np.asarray(bias))

    nc = _build_nc(k_tile, nch)

    in_maps = []
    for c in range(NCORES):
        in_maps.append({"Wext": shared["Wext"],
                        "bias_rep": shared["bias_rep"],
                        "xTs": per_core[c]["xTs"],
                        "bases": per_core[c]["bases"],
                        "selT": per_core[c]["selT"],
                        "selD": per_core[c]["selD"],
                        "zidx": per_core[c]["zidx"]})

    from concourse.bass_utils import run_bass_kernel_spmd
    res = run_bass_kernel_spmd(nc, in_maps, core_ids=list(range(NCORES)),
                               trace=bool(os.environ.get("KERNEL_TRACE")))
    global LAST_RESULTS
    LAST_RESULTS = res
    return np.concatenate([r["out"][:DPC] for r in res.results], 0)


LAST_RESULTS = None


# revision 4
# speedup vs baseline: 1.0221x; 1.0221x over previous
"""GAT-style bipartite graph attention layer (nn_BiGraphContrastLayer) on 8 trn2
cores — v2.2 (549-558 us vs 694 us baseline, rel err 0.0083).

Key facts (measured on HW):
  - DRAM "Shared" scratchpad is shared between core PAIRS {2k, 2k+1} only.
  - SWDGE gathers on one queue drain ~7.7 ns/row; 4 queues give ~390 GB/s.
  - Matmul: 512-col bf16 stream ~380 ns; tiny matmuls floor ~165 ns;
    LDWEIGHTS hides under pipelined streams.
  - Broadcast DVE mult runs ~2x slower than dense; split into 2 chunk-range
    instructions so PE segment-sums can start early.  fp8 for msg/sel was
    tried and REJECTED: rel err 0.065 > 2e-2 gate and no speedup.

Design:
  - Pair-shared zel table [NPAD, 640]: rows = all 10000 src nodes + the
    pair's own 2500 dst nodes (self-loops/era only touch own dsts).  Each
    core computes 50 subtiles of 128 rows (3 uniform runs: 40 src + 9+1 own
    dst, so er of own dsts stays local) at per-core register row offsets
    (dynamic DRAM dma_start).
  - Pairwise AllGather (tiny) as the cross-core barrier.
  - Phase 2: per dst tile, 4 sub-gathers rotated across the 4 SWDGE queues;
    attention + segment-sum matmuls as before.
"""
import os

import numpy as np
import ml_dtypes

import concourse.bacc as bacc
import concourse.bass as bass
import concourse.mybir as mybir
import concourse.tile as tile
from concourse.bass import ds

BF = ml_dtypes.bfloat16
F8 = ml_dtypes.float8_e4m3fn
F32 = np.float32

NS, ND, E, DIN, H, DH = 10000, 10000, 320000, 512, 8, 64
NEG = 0.2
NCORES = 8
DPC = ND // NCORES          # 1250 dst nodes per core
N = NS + ND
NPAD = 20480
ROW = 640                   # zel row elems: z(512) | el(8) | er(8) | pad(112)
NTILES = 10                 # dst tiles of 128 per core (1280 >= 1250)
RUNS = (40, 9, 1)           # src run, dst run, dst tail (x128 rows)
MSUB = sum(RUNS)            # 50


# ----------------------------------------------------------------- host prep
def _wrap_idx(idx):
    k = len(idx)
    w = np.zeros((16, k // 16), np.int16)
    w[np.arange(k) % 16, np.arange(k) // 16] = idx
    return np.tile(w, (8, 1))


def _host_prep(x_src, x_dst, edge_src, edge_dst, W, attn_l, attn_r, bias):
    xcat = np.zeros((NPAD, DIN), F32)
    xcat[:NS] = x_src
    xcat[NS:N] = x_dst
    Al = np.zeros((DIN, H), F32)
    Ar = np.zeros((DIN, H), F32)
    for h in range(H):
        Al[h * DH:(h + 1) * DH, h] = attn_l[h]
        Ar[h * DH:(h + 1) * DH, h] = attn_r[h]
    Wext = np.concatenate([W, W @ Al, W @ Ar], 1).astype(BF)  # [512, 528]
    bias_rep = np.tile(bias[None, :].astype(F32), (128, 1))   # [128, 512]

    edge_src = edge_src.astype(np.int64)
    edge_dst = edge_dst.astype(np.int64)
    tlists = [[None] * NTILES for _ in range(NCORES)]
    kmax = 0
    for c in range(NCORES):
        d0 = c * DPC
        m = (edge_dst >= d0) & (edge_dst < d0 + DPC)
        es = np.concatenate([edge_src[m],
                             NS + d0 + np.arange(DPC, dtype=np.int64)])
        ed = np.concatenate([edge_dst[m] - d0, np.arange(DPC, dtype=np.int64)])
        order = np.argsort(ed, kind="stable")
        es, ed = es[order], ed[order]
        for t in range(NTILES):
            sel = (ed >= t * 128) & (ed < (t + 1) * 128)
            tlists[c][t] = (es[sel], ed[sel] - t * 128)
            kmax = max(kmax, int(sel.sum()))
    k_tile = ((kmax + 127) // 128) * 128
    nch = k_tile // 128

    per_core = []
    for c in range(NCORES):
        zidx = np.zeros((128, NTILES * k_tile // 16), np.int16)
        selT = np.zeros((128, NTILES * nch * 128), BF)
        selD = np.zeros((128, NTILES * nch * 128), BF)
        for t in range(NTILES):
            es, edl = tlists[c][t]
            k = len(es)
            src = np.zeros(k_tile, np.int64)
            src[:k] = es
            s16 = slice(t * k_tile // 16, (t + 1) * k_tile // 16)
            zidx[:, s16] = _wrap_idx(src)
            dstl = np.full(k_tile, -1, np.int64)
            dstl[:k] = edl
            for ch in range(nch):
                dl = dstl[ch * 128:(ch + 1) * 128]
                sm = np.zeros((128, 128), F32)
                valid = dl >= 0
                sm[np.arange(128)[valid], dl[valid]] = 1.0
                j = (t * nch + ch) * 128
                selT[:, j:j + 128] = sm.astype(BF)
                selD[:, j:j + 128] = sm.T.astype(BF)

        # phase-1 shard: 3 runs of subtiles; row bases depend on core parity
        d0 = NS + c * DPC
        if c % 2 == 0:
            bases = [0, d0, d0 + RUNS[1] * 128]
        else:
            bases = [RUNS[0] * 128, d0, d0 + RUNS[1] * 128]
        rows = np.concatenate(
            [np.arange(b, b + r * 128) for b, r in zip(bases, RUNS)])
        rows = np.clip(rows, 0, NPAD - 1)
        xs = xcat[rows]                               # [6400, 512]
        xTs = np.zeros((128, 4, MSUB * 128), BF)
        for k4 in range(4):
            xTs[:, k4, :] = xs[:, k4 * 128:(k4 + 1) * 128].T.astype(BF)
        per_core.append(dict(selT=selT, selD=selD, zidx=zidx,
                             xTs=xTs.reshape(128, 4 * MSUB * 128),
                             bases=np.array([bases], np.int32)))

    shared = dict(Wext=Wext, bias_rep=bias_rep)
    return shared, per_core, k_tile, nch


# ------------------------------------------------------------- bass program
def _build_nc(k_tile, nch):
    nc = bacc.Bacc("TRN2", target_bir_lowering=False, debug=False,
                   num_devices=NCORES, num_swdge_queues=4)
    dt = mybir.dt
    groups = [[2 * p, 2 * p + 1] for p in range(NCORES // 2)]

    W_d = nc.dram_tensor("Wext", [DIN, 528], dt.bfloat16, kind="ExternalInput")
    bias_d = nc.dram_tensor("bias_rep", [128, 512], dt.float32,
                            kind="ExternalInput")
    xTs_d = nc.dram_tensor("xTs", [128, 4 * MSUB * 128], dt.bfloat16,
                           kind="ExternalInput")
    bases_d = nc.dram_tensor("bases", [1, 3], dt.int32, kind="ExternalInput")
    selT_d = nc.dram_tensor("selT", [128, NTILES * nch * 128], dt.bfloat16,
                            kind="ExternalInput")
    selD_d = nc.dram_tensor("selD", [128, NTILES * nch * 128], dt.bfloat16,
                            kind="ExternalInput")
    zidx_d = nc.dram_tensor("zidx", [128, NTILES * k_tile // 16], dt.int16,
                            kind="ExternalInput")
    out_d = nc.dram_tensor("out", [NTILES * 128, 512], dt.float32,
                           kind="ExternalOutput")
    zel_d = nc.dram_tensor("zel_tab", [NPAD, ROW], dt.bfloat16,
                           addr_space="Shared")
    bar_i = nc.dram_tensor("bar_i", [1, 64], dt.int16)
    bar_o = nc.dram_tensor("bar_o", [2, 64], dt.int16)

    with tile.TileContext(nc) as tc:
        with tc.tile_pool(name="const", bufs=1) as cpool:
            wsb = cpool.tile([128, 4 * 528], dt.bfloat16)
            for k in range(4):
                nc.sync.dma_start(wsb[:, k * 528:(k + 1) * 528],
                                  W_d[k * 128:(k + 1) * 128, :])
            bias_sb = cpool.tile([128, 512], dt.float32)
            nc.sync.dma_start(bias_sb[:], bias_d[:])
            zidx_sb = cpool.tile([128, NTILES * k_tile // 16], dt.int16)
            nc.sync.dma_start(zidx_sb[:], zidx_d[:])
            era = cpool.tile([128, NTILES, 8], dt.bfloat16)
            base_sb = cpool.tile([1, 3], dt.int32)
            nc.sync.dma_start(base_sb[:], bases_d[:])
            bar_sb = cpool.tile([1, 64], dt.int16)
            nc.vector.memset(bar_sb[:], 0)
            nc.sync.dma_start(bar_i[:], bar_sb[:])

            rbase = [nc.values_load(base_sb[0:1, i:i + 1], min_val=0,
                                    max_val=NPAD - RUNS[i] * 128,
                                    skip_runtime_bounds_check=True)
                     for i in range(3)]

            # ---- phase 1: this core's zel rows -> pair-shared table
            with (
                tc.tile_pool(name="xp", bufs=1) as xpool,
                tc.tile_pool(name="zel", bufs=3) as zpool,
                tc.tile_pool(name="p1", bufs=2, space="PSUM") as p1pool,
                tc.tile_pool(name="p1b", bufs=2, space="PSUM") as p1bpool,
            ):
                xp = xpool.tile([128, 4, MSUB * 128], dt.bfloat16)
                xv = xTs_d[:].rearrange("p (k m) -> p k m", k=4)
                for j in range(5):
                    s = slice(j * MSUB * 128 // 5, (j + 1) * MSUB * 128 // 5)
                    nc.sync.dma_start(xp[:, :, s], xv[:, :, s])
                m = 0
                for run, base in zip(RUNS, rbase):
                    for i in range(run):
                        zps = p1pool.tile([128, 512], dt.float32,
                                          space="PSUM", name="zps")
                        lps = p1bpool.tile([128, 16], dt.float32,
                                           space="PSUM", name="lps")
                        for k in range(4):
                            lhsT = xp[:, k, m * 128:(m + 1) * 128]
                            nc.tensor.matmul(zps[:], lhsT,
                                             wsb[:, k * 528:k * 528 + 512],
                                             start=(k == 0), stop=(k == 3))
                            nc.tensor.matmul(
                                lps[:], lhsT,
                                wsb[:, k * 528 + 512:(k + 1) * 528],
                                start=(k == 0), stop=(k == 3))
                        zel_sb = zpool.tile([128, ROW], dt.bfloat16,
                                            name="zel_sb")
                        nc.scalar.activation(
                            zel_sb[:, 0:512], zps[:],
                            mybir.ActivationFunctionType.Copy)
                        nc.vector.tensor_copy(zel_sb[:, 512:528], lps[:])
                        if m >= RUNS[0]:
                            nc.vector.tensor_copy(era[:, m - RUNS[0], :],
                                                  lps[:, 8:16])
                        if m < 3:
                            nc.gpsimd.memset(zel_sb[:, 528:ROW], 0)
                        nc.sync.dma_start(zel_d[ds(base + i * 128, 128), :],
                                          zel_sb[:])
                        m += 1

            # ---- pairwise barrier (writes-done -> CC); gathers are held
            # back by a data dep on the CC result instead of a full barrier,
            # so sel/seld loads and er matmuls overlap the CC latency
            tc.strict_bb_all_engine_barrier()
            nc.gpsimd.collective_compute(
                "AllGather", mybir.AluOpType.bypass, groups,
                ins=[bar_i[:].rearrange("a b -> (a b)")],
                outs=[bar_o[:].rearrange("a b -> (a b)")])
            bsb = cpool.tile([1, 64], dt.int16)
            nc.sync.dma_start(bsb[:], bar_o[0:1, :])
            nc.vector.tensor_tensor(
                zidx_sb[0:1, :], zidx_sb[0:1, :],
                bsb[0:1, 0:1].to_broadcast([1, NTILES * k_tile // 16]),
                op=mybir.AluOpType.add)

            # ---- phase 2
            sub = [0]
            for i in range(4):
                sub.append(sub[-1] + (nch + 3 - i) // 4)
            with (
                tc.tile_pool(name="zg", bufs=3) as zgpool,
                tc.tile_pool(name="sel", bufs=2) as selpool,
                tc.tile_pool(name="seld", bufs=2) as seldpool,
                tc.tile_pool(name="sc", bufs=3) as scpool,
                tc.tile_pool(name="eo", bufs=2) as eopool,
                tc.tile_pool(name="p2", bufs=2, space="PSUM") as p2pool,
                tc.tile_pool(name="p2b", bufs=2, space="PSUM") as p2bpool,
                tc.tile_pool(name="p2c", bufs=2, space="PSUM") as p2cpool,
            ):
                for t in range(NTILES):
                    zg = zgpool.tile([128, nch, ROW], dt.bfloat16, name="zg")
                    for j in range(4):
                        c0, c1 = sub[j], sub[j + 1]
                        i16 = slice((t * nch + c0) * 8, (t * nch + c1) * 8)
                        nc.gpsimd.dma_gather(
                            zg[:, c0:c1, :], zel_d[:], zidx_sb[:, i16],
                            num_idxs=(c1 - c0) * 128,
                            num_idxs_reg=(c1 - c0) * 128,
                            elem_size=ROW, single_packet=False,
                            queue_num=j)
                    sel = selpool.tile([128, nch * 128], dt.bfloat16,
                                       name="sel")
                    nc.sync.dma_start(
                        sel[:], selT_d[:, t * nch * 128:(t + 1) * nch * 128])
                    seld = seldpool.tile([128, nch * 128], dt.bfloat16,
                                         name="seld")
                    nc.sync.dma_start(
                        seld[:], selD_d[:, t * nch * 128:(t + 1) * nch * 128])

                    lt = scpool.tile([128, nch, 8], dt.float32, tag="lt",
                                     name="lt")
                    pe_er = p2cpool.tile([128, nch, 8], dt.float32,
                                         space="PSUM", name="pe_er")
                    for ch in range(nch):
                        nc.tensor.matmul(pe_er[:, ch, :],
                                         seld[:, ch * 128:(ch + 1) * 128],
                                         era[:, t, :],
                                         start=True, stop=True,
                                         skip_group_check=True)
                    nc.vector.tensor_tensor(
                        lt[:], zg[:, :, 512:520], pe_er[:],
                        op=mybir.AluOpType.add)
                    nc.vector.scalar_tensor_tensor(
                        lt[:], lt[:], NEG, lt[:],
                        op0=mybir.AluOpType.mult, op1=mybir.AluOpType.max)
                    vb = scpool.tile([128, nch, 8], dt.bfloat16, tag="vb",
                                     name="vb")
                    nc.scalar.activation(vb[:], lt[:],
                                         mybir.ActivationFunctionType.Exp)

                    po = p2pool.tile([128, 512], dt.float32, space="PSUM",
                                     name="po")
                    ps = p2bpool.tile([128, 8], dt.float32, space="PSUM",
                                      name="ps")
                    # msg = v * z split into 2 chunk ranges so segment-sum
                    # matmuls start while the second range still multiplies
                    for j in range(2):
                        c0, c1 = sub[2 * j], sub[2 * j + 2]
                        z4 = zg[:, c0:c1, 0:512].rearrange(
                            "p c (h d) -> p c h d", d=DH)
                        nc.vector.tensor_tensor(
                            z4, z4,
                            vb[:, c0:c1, :].to_broadcast(
                                [128, c1 - c0, 8, DH]),
                            op=mybir.AluOpType.mult)
                        for ch in range(c0, c1):
                            sl = sel[:, ch * 128:(ch + 1) * 128]
                            nc.tensor.matmul(po[:], sl, zg[:, ch, 0:512],
                                             start=(ch == 0),
                                             stop=(ch == nch - 1))
                            nc.tensor.matmul(ps[:], sl, vb[:, ch, :],
                                             start=(ch == 0),
                                             stop=(ch == nch - 1))

                    ssb = scpool.tile([128, 8], dt.float32, tag="ssb",
                                      name="ssb")
                    nc.vector.tensor_scalar_add(ssb[:], ps[:], 1e-30)
                    nc.vector.reciprocal(ssb[:], ssb[:])
                    osb = eopool.tile([128, 512], dt.float32, name="osb")
                    o4 = osb[:].rearrange("p (h d) -> p h d", d=DH)
                    nc.vector.tensor_tensor(
                        o4, po[:].rearrange("p (h d) -> p h d", d=DH),
                        ssb[:].to_broadcast([128, 8, DH]),
                        op=mybir.AluOpType.mult)
                    nc.vector.tensor_tensor(osb[:], osb[:], bias_sb[:],
                                            op=mybir.AluOpType.add)
                    nc.sync.dma_start(out_d[t * 128:(t + 1) * 128, :], osb[:])
    nc.compile()
    return nc


# ------------------------------------------------------------------- driver
def kernel(x_src, x_dst, edge_src, edge_dst, W, attn_l, attn_r, bias):
    shared, per_core, k_tile, nch = _host_prep(
        np.asarray(x_src), np.asarray(x_dst), np.asarray(edge_src),
        np.asarray(edge_dst), np.asarray(W), np.asarray(attn_l),
        np.asarray(attn_r), np.asarray(bias))

    nc = _build_nc(k_tile, nch)

    in_maps = []
    for c in range(NCORES):
        in_maps.append({"Wext": shared["Wext"],
                        "bias_rep": shared["bias_rep"],
                        "xTs": per_core[c]["xTs"],
                        "bases": per_core[c]["bases"],
                        "selT": per_core[c]["selT"],
                        "selD": per_core[c]["selD"],
                        "zidx": per_core[c]["zidx"]})

    from concourse.bass_utils import run_bass_kernel_spmd
    res = run_bass_kernel_spmd(nc, in_maps, core_ids=list(range(NCORES)),
                               trace=bool(os.environ.get("KERNEL_TRACE")))
    global LAST_RESULTS
    LAST_RESULTS = res
    return np.concatenate([r["out"][:DPC] for r in res.results], 0)


LAST_RESULTS = None
